# revision 35
# baseline (speedup 1.0000x reference)
"""KAN layer Trainium2 kernel.

Math: out[b,o] = sum_{i,g} exp(-|tanh(x[b,i]) - grid[g]| * s[o,i]) * w[o,i,g]

Approximation: f_{o,i}(t) is fit per (o,i) in a C0 piecewise-polynomial space
on 3 intervals of t in [-1,1] (cubic, quadratic, cubic), parameterized by
saturating ramps
    r_v(t) = clamp(3*(t - c_v), -1, 1),   c_v in {-2/3, 0, 2/3}
and their powers. Basis columns (K=9): [1, r0, r0^2, r0^3, r1, r1^2,
r2, r2^2, r2^3] - all continuous, so no interval masks or mask*poly products
are needed on device. Coefficients come from a least-squares Chebyshev-node
fit against the exact f, computed on the host from the weights only (fp16
quantization of the basis is folded into the fit). The constant column is
folded into an fp32 bias vector (summed in f64 on the host) that ships as two
fp16-slot rows and is read back via an AP bitcast, applied by the final
PSUM->SBUF add.

Device per core (128 batch rows, data-parallel over 8 cores):
    t = tanh(x)               fp16, Act engine
    ramps + powers            fp16, DVE (4x/2x SBUF perf modes)
    out = Phi @ C + bias      16 accumulated (128x128)@(128x256) fp16 matmuls
Contraction length 8*256 = 2048 vs 12544 for the exact interval-decomposed
Chebyshev formulation; all matmul operands fp16 (4x PE throughput vs fp32,
half the HBM traffic). x rides in the first DMA group with the first 4 weight
chunks so the single DMA engine streams back-to-back with no small transfers.
"""

import numpy as np

B, I, O, G = 1024, 256, 256, 8
N_CORES = 8
BSH = B // N_CORES     # 128 batch rows per core
IH = I // 128          # 2 partition halves of the i dimension
NV = 3
DEGS = (3, 2, 3)       # per-interval polynomial degree (middle is smoother)
NBLK = sum(DEGS)       # 8 non-constant basis functions
NCHUNK = NBLK * IH     # 16 matmul chunks of contraction 128
CENTERS = (-2.0 / 3.0, 0.0, 2.0 / 3.0)
SCALE = 3.0

G0CH = 4               # chunks riding with x in DMA group 0
NWARM = 10             # PE clock-ramp warmup matmuls

_CACHE = {}


def _fit_coeffs(spline_weight, spline_scaler, grid):
    """Least-squares Chebyshev-node fit of f in the ramp-power basis.
    Returns (C[NBLK, O, I] float16, bias[O] float32)."""
    w = spline_weight.astype(np.float64)          # (O, I, G)
    s = spline_scaler.astype(np.float64)          # (O, I)
    g = grid.astype(np.float64)                   # (G,)

    def q16(a):
        return np.asarray(a, np.float32).astype(np.float16).astype(np.float64)

    M = 16
    nodes = np.cos(np.pi * (np.arange(M) + 0.5) / M)
    hw = 1.0 / NV
    tn = (np.asarray(CENTERS)[:, None] + hw * nodes[None, :]).reshape(-1)

    cols = [np.ones_like(tn)]
    for v in range(NV):
        r = q16(np.clip(SCALE * (tn - CENTERS[v]), -1.0, 1.0))
        p = r
        for _ in range(DEGS[v]):
            cols.append(p)
            p = q16(p * r)
    PhiN = np.stack(cols, axis=1)                 # (NV*M, 1+NBLK)
    pinv = np.linalg.pinv(PhiN)

    dist = np.abs(tn[None, None, :, None] - g[None, None, None, :])
    E = np.exp(-dist * s[:, :, None, None])       # (O, I, NV*M, G)
    F = np.einsum('oing,oig->oin', E, w)          # (O, I, NV*M)
    C = np.einsum('kn,oin->koi', pinv, F)         # (1+NBLK, O, I)
    bias = C[0].sum(axis=1).astype(np.float32)    # (O,)
    return C[1:].astype(np.float16), bias


def _pack_dmat(C, bias):
    """-> (g0_d [128, G0CH*O] f16, dmid [2, 128, 6*O] f16, dlast [128, 2*O]).
    Chunks j = 2k+hh; DMA groups: [x|c0..c3], [c4..c9], [c10..c15],
    [bias (f32 rows viewed as 2 f16 slots)]."""
    ch = np.zeros((NCHUNK, 128, O), np.float16)
    for k in range(NBLK):
        for hh in range(IH):
            ch[2 * k + hh] = C[k, :, hh * 128:(hh + 1) * 128].T
    g0_d = np.ascontiguousarray(ch[:G0CH].transpose(1, 0, 2)
                                .reshape(128, G0CH * O))
    mid = [np.ascontiguousarray(ch[a:a + 6].transpose(1, 0, 2)
                                .reshape(128, 6 * O))
           for a in (G0CH, G0CH + 6)]
    bias_rep = np.repeat(bias[None, :], 128, axis=0).astype(np.float32)
    dlast = np.ascontiguousarray(bias_rep.view(np.float16))
    return (g0_d, np.ascontiguousarray(np.stack(mid, axis=0)), dlast)


def _build_module():
    import concourse.bacc as bacc
    import concourse.bass as bass
    import concourse.mybir as mybir
    import concourse.tile as tile

    f32 = mybir.dt.float32
    f16 = mybir.dt.float16
    AF = mybir.ActivationFunctionType
    ALU = mybir.AluOpType

    nc = bacc.Bacc("TRN2", target_bir_lowering=False, debug=False,
                   num_devices=N_CORES)

    xg0 = nc.dram_tensor("xg0", [128, IH * BSH + G0CH * O], f16,
                         kind="ExternalInput")
    dmid = nc.dram_tensor("dmid", [2, 128, 6 * O], f16, kind="ExternalInput")
    dlast = nc.dram_tensor("dlast", [128, 2 * O], f16, kind="ExternalInput")
    out_d = nc.dram_tensor("out", [BSH, O], f16, kind="ExternalOutput")

    with tile.TileContext(nc) as tc:
        with (
            tc.tile_pool(name="keep", bufs=1) as keep,
            tc.tile_pool(name="dbuf", bufs=4) as dbuf,
            tc.tile_pool(name="psum", bufs=1, space=bass.MemorySpace.PSUM) as ppool,
        ):
            xg0_sb = keep.tile([128, IH * BSH + G0CH * O], f16, tag="xg0")
            nc.sync.dma_start(xg0_sb[:], xg0[:])
            x_sb = xg0_sb[:, 0:IH * BSH]

            mid_sb = []
            for g in range(2):
                dt_ = dbuf.tile([128, 6 * O], f16, tag="d", name=f"dsb{g}")
                mid_sb.append(dt_)
                nc.sync.dma_start(dt_[:], dmid[g])
            last_sb = keep.tile([128, 2 * O], f16, tag="dlast")
            nc.sync.dma_start(last_sb[:], dlast[:])

            def rhs_ap(j):
                if j < G0CH:
                    return xg0_sb[:, IH * BSH + j * O:IH * BSH + (j + 1) * O]
                g, m = divmod(j - G0CH, 6)
                return mid_sb[g][:, m * O:(m + 1) * O]

            t = keep.tile([128, IH * BSH], f16, tag="t")
            nc.scalar.activation(t[:], x_sb, AF.Tanh)

            ones = keep.tile([128, IH * BSH], f16, tag="ones")
            nc.gpsimd.memset(ones[:], 1.0)

            # saturating ramps (2 fused tensor_scalar each) + powers, all fp16
            blocks = []
            for v in range(NV):
                r = keep.tile([128, IH * BSH], f16, tag=f"r{v}")
                nc.vector.tensor_scalar(r[:], t[:], SCALE, -SCALE * CENTERS[v],
                                        ALU.mult, ALU.add)
                nc.vector.tensor_scalar(r[:], r[:], 1.0, -1.0, ALU.min, ALU.max)
                blocks.append(r)
                p = r
                for _ in range(1, DEGS[v]):
                    p2 = keep.tile([128, IH * BSH], f16, tag=f"p{len(blocks)}")
                    nc.vector.tensor_tensor(p2[:], p[:], r[:], ALU.mult)
                    blocks.append(p2)
                    p = p2
            assert len(blocks) == NBLK

            # PE clock-ramp warmup on junk data
            wpsum = ppool.tile([BSH, O], f32, tag="warm")
            for _ in range(NWARM):
                nc.tensor.matmul(wpsum[:], ones[:, 0:BSH], ones[:],
                                 start=True, stop=True)

            acc = ppool.tile([BSH, O], f32, tag="acc")
            for j in range(NCHUNK):
                k, hh = divmod(j, 2)
                lhsT = blocks[k][:, hh * BSH:(hh + 1) * BSH]
                nc.tensor.matmul(acc[:], lhsT, rhs_ap(j),
                                 start=(j == 0), stop=(j == NCHUNK - 1))

            bias_ap = last_sb[:].bitcast(f32)
            osb = keep.tile([BSH, O], f16, tag="o")
            nc.vector.tensor_tensor(osb[:], acc[:], bias_ap, ALU.add)
            nc.sync.dma_start(out_d[:], osb[:])

    nc.compile()
    return nc


def kernel(x, spline_weight, spline_scaler, grid):
    from concourse import bass_utils

    C, bias = _fit_coeffs(np.asarray(spline_weight), np.asarray(spline_scaler),
                          np.asarray(grid))
    g0_d, dmid, dlast = _pack_dmat(C, bias)

    if "nc" not in _CACHE:
        _CACHE["nc"] = _build_module()
    nc = _CACHE["nc"]

    x = np.asarray(x, dtype=np.float32)
    in_maps = []
    for cid in range(N_CORES):
        xsT = x[cid * BSH:(cid + 1) * BSH].T.astype(np.float16)   # (I, BSH)
        xdv = np.concatenate([xsT[:128], xsT[128:]], axis=1)
        xg0v = np.ascontiguousarray(np.concatenate([xdv, g0_d], axis=1))
        in_maps.append({"xg0": xg0v, "dmid": dmid, "dlast": dlast})

    import os
    trace = bool(int(os.environ.get("KAN_TRACE", "0")))
    kw = {}
    if trace:
        tdir = os.environ.get("KAN_TRACE_DIR") or None
        kw = dict(trace=True, tmpdir=tdir)
    res = bass_utils.run_bass_kernel_spmd(nc, in_maps,
                                          core_ids=list(range(N_CORES)), **kw)
    _CACHE["last_result"] = res
    out = np.concatenate([res.results[cid]["out"] for cid in range(N_CORES)], axis=0)
    return out.astype(np.float32)
